# revision 1
# baseline (speedup 1.0000x reference)
"""Trainium2 Bass kernel for nn_Net_14869176779172 (moe_routing).

Computes, for x[B=1024, D=4096, S=60], W[D, S], soma_w[D], soma_b[1]:
    d[b, j]  = sum_s x[b, j, s] * W[j, s]          (per-dendrite dot)
    r        = relu(d)
    act[b,j] = sigmoid(r)        for j < 1638      (first 40% of dendrites)
             = sqrt(r)           otherwise
    out[b]   = act[b, :] @ soma_w + soma_b         -> [B, 1]

Sharding: pure data-parallel over batch across 8 NeuronCores (128 batch
rows per core); W / soma_w / soma_b replicated.

Per-core layout: batch on the 128 SBUF partitions (so all x DMAs are
fully contiguous per partition).  W is broadcast across partitions with
a ones[1,128] matmul on the (otherwise idle) TensorEngine into PSUM;
the VectorEngine does the x*W multiply (reading W from PSUM) and the
segmented reduction over S.  ScalarEngine applies sigmoid/sqrt.
"""

import numpy as np

import concourse.bacc as bacc
import concourse.bass as bass
import concourse.tile as tile
from concourse import mybir
from concourse.bass_utils import run_bass_kernel_spmd

# Problem constants (hardcoded per harness contract).
B_TOTAL = 1024
N_CORES = 8
B = B_TOTAL // N_CORES  # 128 batch rows per core
D = 4096
S = 60
CUT = int(D * 0.4)  # 1638: first CUT dendrites use sigmoid, rest sqrt

P = 128  # SBUF partitions

# Tiling: a "piece" is 32 dendrites (32*60 = 1920 floats per partition),
# whose broadcast weights fit in 4 PSUM banks ([128, 4, 512] fp32).
PIECE_D = 32
PIECE_F = PIECE_D * S  # 1920
N_PIECES = D // PIECE_D  # 128
# x is DMA'd in chunks of 4 pieces (128 dendrites, 3.9 MB per DMA).
CHUNK_PIECES = 4
CHUNK_F = CHUNK_PIECES * PIECE_F  # 7680
CHUNK_D = CHUNK_PIECES * PIECE_D  # 128
N_CHUNKS = D // CHUNK_D  # 32

FP32 = mybir.dt.float32
FP16 = mybir.dt.float16


def _build_program():
    nc = bacc.Bacc(
        "TRN2",
        target_bir_lowering=False,
        debug=False,
        enable_asserts=False,
        num_devices=N_CORES,
    )

    x_d = nc.dram_tensor("x", [B, D, S], FP32, kind="ExternalInput")
    w_d = nc.dram_tensor("W16", [D, S], FP16, kind="ExternalInput")
    sw_d = nc.dram_tensor("soma_w", [D], FP32, kind="ExternalInput")
    sb_d = nc.dram_tensor("soma_b", [1], FP32, kind="ExternalInput")
    oh_d = nc.dram_tensor("onehot", [32, 32 * P], FP16, kind="ExternalInput")
    out_d = nc.dram_tensor("out", [B, 1], FP32, kind="ExternalOutput")

    x_flat = x_d.ap().rearrange("b d s -> b (d s)")  # [128, 245760]
    w_flat = w_d.ap().rearrange("d s -> (d s)")  # [245760]

    with tile.TileContext(nc) as tc:
        with (
            tc.tile_pool(name="singles", bufs=1) as singles,
            tc.tile_pool(name="xpool", bufs=4) as xpool,
            tc.tile_pool(name="ypool", bufs=2) as ypool,
            tc.tile_pool(name="y2pool", bufs=2) as y2pool,
            tc.tile_pool(name="yspool", bufs=3) as yspool,
            tc.tile_pool(name="w16pool", bufs=2) as w16pool,
            tc.tile_pool(name="wpsum", bufs=2, space="PSUM") as wpsum,
        ):
            # ---- constants / small inputs ----
            # One-hot row-selector stationaries: onehot_t[:, r*128:(r+1)*128]
            # is a [32, 128] matrix whose row r is all-ones.  Matmul with it
            # broadcasts partition r of the rhs to all 128 output partitions.
            # (Supplied from the host: engines can't memset at partition r>0.)
            onehot_t = singles.tile([32, 32 * P], FP16)
            nc.sync.dma_start(out=onehot_t, in_=oh_d.ap())

            # W, reshaped [32, 7680]: partition p holds dendrite pieces
            # 4p..4p+3 (each piece = 32 dendrites * 60 syn = 1920 floats).
            w_sb = singles.tile([32, 4 * PIECE_F], FP16)
            nc.sync.dma_start(
                out=w_sb, in_=w_flat.rearrange("(p f) -> p f", p=32)
            )

            # soma_w broadcast to all partitions: [128, 4096] (2 MB DMA).
            swb = singles.tile([P, D], FP32)
            sw_ap = sw_d.ap()
            nc.sync.dma_start(
                out=swb,
                in_=bass.AP(
                    tensor=sw_ap.tensor, offset=sw_ap.offset, ap=[[0, P], *sw_ap.ap]
                ),
            )
            # soma_b broadcast: [128, 1]
            sbb = singles.tile([P, 1], FP32)
            sb_ap = sb_d.ap()
            nc.sync.dma_start(
                out=sbb,
                in_=bass.AP(
                    tensor=sb_ap.tensor, offset=sb_ap.offset, ap=[[0, P], *sb_ap.ap]
                ),
            )

            # accumulator for all dendrite outputs of this core's batch rows
            z_buf = singles.tile([P, D], FP32)
            # running partial sums for the soma dot product
            acc = singles.tile([P, P], FP32)
            nc.vector.memset(acc, 0.0)

            # ---- main loop: multiply + segmented reduce ----
            for c in range(N_CHUNKS):
                # x chunk, cast fp32 -> fp16 during the DMA (SWDGE)
                xc = xpool.tile([P, CHUNK_F], FP16)
                nc.gpsimd.dma_start(
                    out=xc, in_=x_flat[:, c * CHUNK_F : (c + 1) * CHUNK_F]
                )
                # Broadcast this chunk's W across partitions via TensorE,
                # evacuating PSUM -> SBUF (fp16) on ScalarE piece by piece.
                w16 = w16pool.tile([P, CHUNK_F], FP16)
                for k in range(CHUNK_PIECES):
                    pi = c * CHUNK_PIECES + k
                    p_row, g = pi // 4, pi % 4
                    wb = wpsum.tile([P, 4, 512], FP32)
                    for j in range(4):
                        nc.tensor.matmul(
                            wb[:, j, 0:480],
                            onehot_t[:, p_row * P : (p_row + 1) * P],
                            w_sb[:, g * PIECE_F + j * 480 : g * PIECE_F + (j + 1) * 480],
                        )
                    nc.scalar.copy(
                        out=w16[:, k * PIECE_F : (k + 1) * PIECE_F].rearrange(
                            "p (a f) -> p a f", a=4
                        ),
                        in_=wb[:, :, 0:480],
                    )
                # one 2x-mode multiply for the whole chunk
                y = ypool.tile([P, CHUNK_F], FP16)
                nc.vector.tensor_mul(y, xc, w16)
                # first reduction step y2[d,s] = y[d,s] + y[d,s+30]; alternate
                # chunks go to GpSimd to offload the VectorE
                y3 = y.rearrange("p (d s) -> p d s", s=S)
                y2 = y2pool.tile([P, CHUNK_D * (S // 2)], FP16)
                half_eng = nc.gpsimd if (c % 2 == 0) else nc.vector
                half_eng.tensor_add(
                    y2.rearrange("p (d s) -> p d s", s=S // 2),
                    y3[:, :, 0 : S // 2],
                    y3[:, :, S // 2 : S],
                )
                nc.vector.tensor_reduce(
                    out=z_buf[:, c * CHUNK_D : (c + 1) * CHUNK_D],
                    in_=y2.rearrange("p (d s) -> p d s", s=S // 2),
                    axis=mybir.AxisListType.X,
                    op=mybir.AluOpType.add,
                )

                # ---- per-chunk activations + soma partial (overlap the tail)
                d0, d1 = c * CHUNK_D, (c + 1) * CHUNK_D
                zc = z_buf[:, d0:d1]
                nc.vector.tensor_scalar_max(out=zc, in0=zc, scalar1=0.0)
                # sigmoid region [0, CUT), sqrt region [CUT, D)
                if d1 <= CUT:
                    nc.scalar.activation(
                        out=zc, in_=zc, func=mybir.ActivationFunctionType.Sigmoid
                    )
                elif d0 >= CUT:
                    nc.scalar.activation(
                        out=zc, in_=zc, func=mybir.ActivationFunctionType.Sqrt
                    )
                else:
                    nc.scalar.activation(
                        out=z_buf[:, d0:CUT],
                        in_=z_buf[:, d0:CUT],
                        func=mybir.ActivationFunctionType.Sigmoid,
                    )
                    nc.scalar.activation(
                        out=z_buf[:, CUT:d1],
                        in_=z_buf[:, CUT:d1],
                        func=mybir.ActivationFunctionType.Sqrt,
                    )
                # acc[:, j] += sum over this chunk's dendrite groups of act*soma_w
                ysc = yspool.tile([P, CHUNK_D], FP32)
                nc.vector.tensor_mul(ysc, zc, swb[:, d0:d1])
                for g in range(CHUNK_D // P):
                    nc.vector.tensor_add(acc, acc, ysc[:, g * P : (g + 1) * P])

            # ---- soma: out = sum(acc) + soma_b ----
            zsum = singles.tile([P, 1], FP32)
            nc.vector.tensor_reduce(
                out=zsum,
                in_=acc,
                axis=mybir.AxisListType.X,
                op=mybir.AluOpType.add,
            )
            out_sb = singles.tile([P, 1], FP32)
            nc.vector.tensor_add(out_sb, zsum, sbb)
            nc.sync.dma_start(out=out_d.ap().rearrange("b one -> b one"), in_=out_sb)

    nc.compile()
    return nc


_NC_CACHE = None


def _get_program():
    global _NC_CACHE
    if _NC_CACHE is None:
        _NC_CACHE = _build_program()
    return _NC_CACHE


def kernel(x, W, soma_w, soma_b, _trace=False):
    nc = _get_program()
    x = np.ascontiguousarray(x, dtype=np.float32)
    W = np.ascontiguousarray(W, dtype=np.float32)
    soma_w = np.ascontiguousarray(soma_w, dtype=np.float32)
    soma_b = np.ascontiguousarray(soma_b, dtype=np.float32)

    onehot = np.ascontiguousarray(
        np.repeat(np.eye(32, dtype=np.float16), P, axis=1)
    )  # [32, 32*128]
    in_maps = [
        {
            "x": np.ascontiguousarray(x[i * B : (i + 1) * B]),
            "W16": W.astype(np.float16),
            "soma_w": soma_w,
            "soma_b": soma_b,
            "onehot": onehot,
        }
        for i in range(N_CORES)
    ]
    res = run_bass_kernel_spmd(
        nc, in_maps, core_ids=list(range(N_CORES)), trace=_trace
    )
    out = np.concatenate([r["out"] for r in res.results], axis=0)
    if _trace:
        kernel.last_results = res
    return out.astype(np.float32)



# revision 7
# speedup vs baseline: 1.0070x; 1.0070x over previous
"""Trainium2 Bass kernel for nn_Net_14869176779172 (moe_routing).

Computes, for x[B=1024, D=4096, S=60], W[D, S], soma_w[D], soma_b[1]:
    d[b, j]  = sum_s x[b, j, s] * W[j, s]          (per-dendrite dot)
    r        = relu(d)
    act[b,j] = sigmoid(r)        for j < 1638      (first 40% of dendrites)
             = sqrt(r)           otherwise
    out[b]   = act[b, :] @ soma_w + soma_b         -> [B, 1]

Sharding: pure data-parallel over batch across 8 NeuronCores (128 batch
rows per core); W / soma_w / soma_b replicated.

Per-core layout (v2): partition p = jblk*4 + bidx, where jblk in [0,32)
indexes a block of 128 dendrites and bidx in [0,4) a batch row within a
4-row group.  W is replicated only 4x (host-prepped [128, 7680] fp16
tile, resident in SBUF all kernel), so there is NO per-chunk W broadcast
machinery.  x streams in 32 cast-DMAs (fp32->fp16, 3.9 MB each, 30 KB
contiguous per partition line); DVE does an in-place multiply (2x mode)
+ segmented reduce per group.  Both sigmoid and sqrt are applied
full-width on ScalarE; region selection is folded into two masked
soma-weight tensors (avoids partition-sliced ops, which the BIR
verifier rejects off 32-partition boundaries).  The final
cross-partition soma sum (over jblk) is one matmul with a selector.
"""

import numpy as np

import concourse.bacc as bacc
import concourse.bass as bass
import concourse.tile as tile
from concourse import mybir
from concourse.bass_utils import run_bass_kernel_spmd

# Problem constants (hardcoded per harness contract).
B_TOTAL = 1024
N_CORES = 8
B = B_TOTAL // N_CORES  # 128 batch rows per core
D = 4096
S = 60
CUT = int(D * 0.4)  # 1638: first CUT dendrites use sigmoid, rest sqrt

P = 128  # SBUF partitions
NJ = 32  # dendrite blocks
DJ = D // NJ  # 128 dendrites per block
NB = 4  # batch rows per group (per DMA)
NG = B // NB  # 32 groups
GF = DJ * S  # 7680 elems per partition per group
ROW_F = D * S  # 245760 elems per batch row

FP32 = mybir.dt.float32
FP16 = mybir.dt.float16


def _build_program():
    nc = bacc.Bacc(
        "TRN2",
        target_bir_lowering=False,
        debug=False,
        enable_asserts=False,
        num_devices=N_CORES,
    )

    x_d = nc.dram_tensor("x", [B, D, S], FP32, kind="ExternalInput")
    w2_d = nc.dram_tensor("W2", [P, GF], FP16, kind="ExternalInput")
    # packed small inputs (the NEFF loader rejects >5 input tensors):
    # SWPAIR = [sw_sig | sw_sqrt] fp16; SELSB = [sel | soma_b] fp32
    swpair_d = nc.dram_tensor("SWPAIR", [P, 2 * DJ], FP16, kind="ExternalInput")
    selsb_d = nc.dram_tensor("SELSB", [P, NB + 1], FP32, kind="ExternalInput")
    out_d = nc.dram_tensor("out", [B, 1], FP32, kind="ExternalOutput")

    x_ap = x_d.ap().rearrange("b d s -> b (d s)")  # [128, 245760]

    with tile.TileContext(nc) as tc:
        with (
            tc.tile_pool(name="singles", bufs=1) as singles,
            tc.tile_pool(name="xpool", bufs=6) as xpool,
            tc.tile_pool(name="zpool", bufs=3) as zpool,
            tc.tile_pool(name="zspool", bufs=3) as zspool,
            tc.tile_pool(name="scrpool", bufs=2) as scrpool,
            tc.tile_pool(name="psum", bufs=1, space="PSUM") as psum_pool,
        ):
            # ---- resident small tensors (HWDGE loads, overlap with x DMA) ----
            w2 = singles.tile([P, GF], FP16)
            nc.sync.dma_start(out=w2, in_=w2_d.ap())
            sw_pair = singles.tile([P, 2 * DJ], FP16)
            nc.sync.dma_start(out=sw_pair, in_=swpair_d.ap())
            sw_sig = sw_pair[:, 0:DJ]
            sw_sqrt = sw_pair[:, DJ : 2 * DJ]
            selsb = singles.tile([P, NB + 1], FP32)
            nc.sync.dma_start(out=selsb, in_=selsb_d.ap())
            sel = selsb[:, 0:NB]
            sbb = selsb[:, NB : NB + 1]

            # per-(partition, group) soma partial sums: [:, g] sigmoid-region,
            # [:, NG+g] sqrt-region
            acc = singles.tile([P, 2 * NG], FP32)

            for g in range(NG):
                # x group: partition p=(jblk,bidx) reads batch row g*NB+bidx,
                # dendrites [jblk*DJ, (jblk+1)*DJ) -- 30 KB contiguous fp32,
                # cast to fp16 in the SDMA datapath (SWDGE).
                xt = xpool.tile([P, GF], FP16)
                src = bass.AP(
                    tensor=x_ap.tensor,
                    offset=x_ap.offset + g * NB * ROW_F,
                    ap=[[GF, NJ], [ROW_F, NB], [1, GF]],
                )
                nc.gpsimd.dma_start(out=xt, in_=src)

                # y = x * W (in-place, fp16 2x mode)
                nc.vector.tensor_mul(xt, xt, w2)

                # z[p, c] = sum_s y[p, c, s]  (fp16 out keeps DVE in 2x mode)
                zg = zpool.tile([P, DJ], FP16)
                with nc.allow_low_precision(
                    "fp16 dendrite sums; tol 2e-2, DVE accumulates fp32 internally"
                ):
                    nc.vector.tensor_reduce(
                        out=zg,
                        in_=xt.rearrange("p (c s) -> p c s", s=S),
                        axis=mybir.AxisListType.X,
                        op=mybir.AluOpType.add,
                    )

                # r = relu(z); zs = sigmoid(r); zg <- sqrt(r)   (all ScalarE)
                nc.scalar.activation(
                    out=zg, in_=zg, func=mybir.ActivationFunctionType.Relu
                )
                zs = zspool.tile([P, DJ], FP16)
                nc.scalar.activation(
                    out=zs, in_=zg, func=mybir.ActivationFunctionType.Sigmoid
                )
                nc.scalar.activation(
                    out=zg, in_=zg, func=mybir.ActivationFunctionType.Sqrt
                )

                # acc[p, g] = sum_c zs*sw_sig;  acc[p, NG+g] = sum_c zq*sw_sqrt
                # (tensor_tensor_reduce crashes the runtime; use mul+reduce)
                scr = scrpool.tile([P, DJ], FP16)
                nc.vector.tensor_mul(scr, zs, sw_sig)
                nc.vector.tensor_reduce(
                    out=acc[:, g : g + 1],
                    in_=scr,
                    axis=mybir.AxisListType.X,
                    op=mybir.AluOpType.add,
                )
                scr2 = scrpool.tile([P, DJ], FP16)
                nc.vector.tensor_mul(scr2, zg, sw_sqrt)
                nc.vector.tensor_reduce(
                    out=acc[:, NG + g : NG + g + 1],
                    in_=scr2,
                    axis=mybir.AxisListType.X,
                    op=mybir.AluOpType.add,
                )

            # ---- final: out[g*NB+i] = sum_jblk (acc_sig + acc_sqrt) + soma_b
            ps = psum_pool.tile([NB, 2 * NG], FP32)
            nc.tensor.matmul(ps, sel, acc)
            ps_sb = singles.tile([NB, 2 * NG], FP32)
            nc.scalar.copy(out=ps_sb, in_=ps)
            out_sb = singles.tile([NB, NG], FP32)
            nc.vector.tensor_add(out_sb, ps_sb[:, 0:NG], ps_sb[:, NG : 2 * NG])
            nc.vector.tensor_scalar_add(out=out_sb, in0=out_sb, scalar1=sbb[0:NB])
            oa = out_d.ap().rearrange("b one -> (b one)")
            nc.sync.dma_start(
                out=bass.AP(tensor=oa.tensor, offset=oa.offset, ap=[[1, NB], [NB, NG]]),
                in_=out_sb,
            )

    nc.compile()
    return nc


_NC_CACHE = None


def _get_program():
    global _NC_CACHE
    if _NC_CACHE is None:
        _NC_CACHE = _build_program()
    return _NC_CACHE


def kernel(x, W, soma_w, soma_b, _trace=False):
    nc = _get_program()
    x = np.ascontiguousarray(x, dtype=np.float32)
    W16 = np.asarray(W, dtype=np.float16)
    soma_w = np.asarray(soma_w, dtype=np.float32)
    soma_b = np.asarray(soma_b, dtype=np.float32)

    w2 = np.ascontiguousarray(np.repeat(W16.reshape(NJ, GF), NB, axis=0))
    sw16 = soma_w.astype(np.float16)
    is_sig = np.arange(D) < CUT
    sw_sig = np.repeat(np.where(is_sig, sw16, 0).reshape(NJ, DJ), NB, axis=0)
    sw_sqrt = np.repeat(np.where(is_sig, 0, sw16).reshape(NJ, DJ), NB, axis=0)
    sw_pair = np.ascontiguousarray(np.concatenate([sw_sig, sw_sqrt], axis=1))
    sel = (np.arange(P)[:, None] % NB == np.arange(NB)[None, :]).astype(np.float32)
    sb = np.full((P, 1), float(soma_b.reshape(-1)[0]), np.float32)
    selsb = np.ascontiguousarray(np.concatenate([sel, sb], axis=1))

    in_maps = [
        {
            "x": x[i * B : (i + 1) * B],
            "W2": w2,
            "SWPAIR": sw_pair,
            "SELSB": selsb,
        }
        for i in range(N_CORES)
    ]
    res = run_bass_kernel_spmd(
        nc, in_maps, core_ids=list(range(N_CORES)), trace=_trace
    )
    out = np.concatenate([r["out"] for r in res.results], axis=0)
    if _trace:
        kernel.last_results = res
    return out.astype(np.float32)
